# revision 15
# baseline (speedup 1.0000x reference)
"""4x4 array-multiplier kernel for Trainium2 (Bass/Tile), 8-core SPMD.

The reference nn.Module is a spiking-neuron gate network implementing a
combinational 4x4 binary multiplier: A, B are [N, 4] float32 bit vectors
(LSB first), output is [N, 8] float32 bits of the product.

Closed form used here (exact in bf16/f32 since all values are small
integers):
    a = A0 + 2*A1 + 4*A2 + 8*A3          (0..15)
    b = B0 + 2*B1 + 4*B2 + 8*B3
    p = a * b                             (0..225)
    out_k = bit k of p, via a compare-subtract chain from the MSB.

Per-core layout (N/8 rows per core, pure data parallel, no comms):
  - DMA in A,B tiles [128, f, 4] f32 (contiguous rows per partition).
  - ScalarE (ACT) deinterleaves bit j to a bf16 plane scaled by 2^j
    (Copy activation with scale) - runs on the otherwise-idle ACT engine.
  - DVE: tt-add tree for a,b; one bf16 multiply for p; then
    bit_k = (r >= 2^k) written to a contiguous bf16 bit-plane and
    r -= 2^k * bit_k (fused scalar_tensor_tensor), k = 7..1;
    bit_0 = A0*B0 directly from the input planes.
  - One DMA out per tile: [128, 8, f] bf16 bit-planes.
Host side: transpose planes to [R, 8] and convert to f32 (bits are
exactly 0.0/1.0, so the conversion is exact).
"""

import os
import sys
from contextlib import ExitStack

import numpy as np

for _p in ("/opt/trn_rl_repo",):
    if _p not in sys.path and os.path.isdir(_p):
        sys.path.insert(0, _p)

import concourse.bass as bass
import concourse.tile as tile
from concourse import bacc, mybir
from concourse.bass_utils import run_bass_kernel_spmd

N_FULL = 4 * 1024 * 1024
N_CORES = 8
R = N_FULL // N_CORES          # rows per core = 524288
F = 1024                       # rows per SBUF partition per tile
ROWS_PER_TILE = 128 * F        # 131072
ALU = mybir.AluOpType
AF = mybir.ActivationFunctionType
F32 = mybir.dt.float32
BF16 = mybir.dt.bfloat16
U8 = mybir.dt.uint8


def emit_multiplier(ctx: ExitStack, tc: "tile.TileContext", Ah, Bh, Oh, rows: int, f: int):
    nc = tc.nc
    rows_per_tile = 128 * f
    nt = rows // rows_per_tile
    assert rows % rows_per_tile == 0

    Av = Ah[:].rearrange("(n p f) c -> n p f c", p=128, f=f)
    Bv = Bh[:].rearrange("(n p f) c -> n p f c", p=128, f=f)
    Ov = Oh[:].rearrange("(n p c f) -> n p c f", p=128, c=8, f=f)

    io_pool = ctx.enter_context(tc.tile_pool(name="io", bufs=2))
    pl_pool = ctx.enter_context(tc.tile_pool(name="planes", bufs=2))
    tmp_pool = ctx.enter_context(tc.tile_pool(name="tmp", bufs=2))

    for i in range(nt):
        At = io_pool.tile([128, f, 4], F32, tag="A")
        Bt = io_pool.tile([128, f, 4], F32, tag="B")
        nc.sync.dma_start(At[:], Av[i])
        nc.sync.dma_start(Bt[:], Bv[i])

        # Deinterleave input bit j into a bf16 plane pre-scaled by 2^j.
        # Slots 0..3 = A0..A3 (scaled 1,2,4,8); 4..7 = B0..B3.
        Dp = pl_pool.tile([128, 8, f], BF16, tag="D")
        for j in range(4):
            # one plane per input on DVE to offload ACT slightly
            eng = nc.vector if j == 0 else nc.scalar
            if j == 0:
                nc.gpsimd.tensor_copy(Dp[:, 0, :], At[:, :, 0])
                nc.gpsimd.tensor_copy(Dp[:, 4, :], Bt[:, :, 0])
            else:
                s = float(2 ** j)
                nc.scalar.activation(Dp[:, j, :], At[:, :, j], AF.Copy, bias=0.0, scale=s)
                nc.scalar.activation(Dp[:, 4 + j, :], Bt[:, :, j], AF.Copy, bias=0.0, scale=s)

        # a,b via tt-add tree on pre-scaled planes (all bf16, exact)
        u = tmp_pool.tile([128, f], BF16, tag="u")
        v = tmp_pool.tile([128, f], BF16, tag="v")
        a = tmp_pool.tile([128, f], BF16, tag="a")
        nc.vector.tensor_tensor(u[:], Dp[:, 0, :], Dp[:, 1, :], ALU.add)
        nc.vector.tensor_tensor(v[:], Dp[:, 2, :], Dp[:, 3, :], ALU.add)
        nc.vector.tensor_tensor(a[:], u[:], v[:], ALU.add)
        u2 = tmp_pool.tile([128, f], BF16, tag="u2")
        v2 = tmp_pool.tile([128, f], BF16, tag="v2")
        b = tmp_pool.tile([128, f], BF16, tag="b")
        nc.vector.tensor_tensor(u2[:], Dp[:, 4, :], Dp[:, 5, :], ALU.add)
        nc.vector.tensor_tensor(v2[:], Dp[:, 6, :], Dp[:, 7, :], ALU.add)
        nc.vector.tensor_tensor(b[:], u2[:], v2[:], ALU.add)

        p = tmp_pool.tile([128, f], BF16, tag="p")
        nc.vector.tensor_mul(p[:], a[:], b[:])

        # bits 7..1: compare-subtract chain, contiguous bf16 planes out
        Pt = io_pool.tile([128, 8, f], U8, tag="O")
        r = p
        for k in range(7, 0, -1):
            nc.vector.tensor_scalar(Pt[:, k, :], r[:], float(2 ** k), None, ALU.is_ge)
            if k > 1:
                rn = tmp_pool.tile([128, f], BF16, tag=f"r{k % 2}")
                nc.vector.scalar_tensor_tensor(
                    rn[:], Pt[:, k, :], float(-(2 ** k)), r[:], ALU.mult, ALU.add
                )
                r = rn
        # bit 0 = A0 AND B0 = A0*B0 (planes 0 and 4 are unscaled)
        nc.vector.tensor_mul(Pt[:, 0, :], Dp[:, 0, :], Dp[:, 4, :])
        nc.sync.dma_start(Ov[i], Pt[:])


def build(rows: int = R, f: int = F) -> bass.Bass:
    nc = bacc.Bacc()
    Ah = nc.declare_dram_parameter("A", [rows, 4], F32, isOutput=False)
    Bh = nc.declare_dram_parameter("B", [rows, 4], F32, isOutput=False)
    Oh = nc.declare_dram_parameter("O", [rows * 8], U8, isOutput=True)
    with tile.TileContext(nc) as tc:
        with ExitStack() as ctx:
            emit_multiplier(ctx, tc, Ah, Bh, Oh, rows, f)
    nc.finalize()
    return nc


def _run(A: np.ndarray, B: np.ndarray, trace: bool = False, tmpdir: str | None = None):
    A = np.ascontiguousarray(np.asarray(A), dtype=np.float32)
    B = np.ascontiguousarray(np.asarray(B), dtype=np.float32)
    assert A.shape == (N_FULL, 4) and B.shape == (N_FULL, 4), (A.shape, B.shape)

    nc = build(R, F)
    in_maps = [
        {"A": A[i * R:(i + 1) * R], "B": B[i * R:(i + 1) * R]}
        for i in range(N_CORES)
    ]
    kres = run_bass_kernel_spmd(
        nc, in_maps, list(range(N_CORES)), trace=trace, tmpdir=tmpdir
    )
    out = np.empty((N_FULL, 8), dtype=np.float32)
    nt = R // ROWS_PER_TILE
    for i in range(N_CORES):
        planes = kres.results[i]["O"].reshape(nt, 128, 8, F)
        # [nt, 128, 8, f] -> rows (nt, 128, f) x bits
        shard = np.transpose(planes, (0, 1, 3, 2)).reshape(R, 8)
        out[i * R:(i + 1) * R] = shard  # bf16 -> f32, exact for 0/1
    return out, kres


def kernel(A: np.ndarray, B: np.ndarray) -> np.ndarray:
    out, _ = _run(A, B, trace=False)
    return out


# revision 16
# speedup vs baseline: 1.1664x; 1.1664x over previous
"""4x4 array-multiplier kernel for Trainium2 (Bass/Tile), 8-core SPMD.

The reference nn.Module is a spiking-neuron gate network implementing a
combinational 4x4 binary multiplier: A, B are [N, 4] float32 bit vectors
(LSB first), output is [N, 8] float32 bits of the product.

Closed form used here (exact in bf16/f32 since all values are small
integers):
    a = A0 + 2*A1 + 4*A2 + 8*A3          (0..15)
    b = B0 + 2*B1 + 4*B2 + 8*B3
    p = a * b                             (0..225)
    out_k = bit k of p, via a compare-subtract chain from the MSB.

Per-core layout (N/8 rows per core, pure data parallel, no comms):
  - DMA in A,B tiles [128, f, 4] f32 (contiguous rows per partition).
  - ScalarE (ACT) deinterleaves bit j to a bf16 plane scaled by 2^j
    (Copy activation with scale) - runs on the otherwise-idle ACT engine.
  - DVE: tt-add tree for a,b; one bf16 multiply for p; then
    bit_k = (r >= 2^k) written to a contiguous bf16 bit-plane and
    r -= 2^k * bit_k (fused scalar_tensor_tensor), k = 7..1;
    bit_0 = A0*B0 directly from the input planes.
  - One DMA out per tile: [128, 8, f] bf16 bit-planes.
Host side: transpose planes to [R, 8] and convert to f32 (bits are
exactly 0.0/1.0, so the conversion is exact).
"""

import os
import sys
from contextlib import ExitStack

import numpy as np

for _p in ("/opt/trn_rl_repo",):
    if _p not in sys.path and os.path.isdir(_p):
        sys.path.insert(0, _p)

import concourse.bass as bass
import concourse.tile as tile
from concourse import bacc, mybir
from concourse.bass_utils import run_bass_kernel_spmd

N_FULL = 4 * 1024 * 1024
N_CORES = 8
R = N_FULL // N_CORES          # rows per core = 524288
F = 1024                       # rows per SBUF partition per tile
ROWS_PER_TILE = 128 * F        # 131072
ALU = mybir.AluOpType
AF = mybir.ActivationFunctionType
F32 = mybir.dt.float32
BF16 = mybir.dt.bfloat16


def emit_multiplier(ctx: ExitStack, tc: "tile.TileContext", Ah, Bh, Oh, rows: int, f: int):
    nc = tc.nc
    rows_per_tile = 128 * f
    nt = rows // rows_per_tile
    assert rows % rows_per_tile == 0

    Av = Ah[:].rearrange("(n p f) c -> n p f c", p=128, f=f)
    Bv = Bh[:].rearrange("(n p f) c -> n p f c", p=128, f=f)
    Ov = Oh[:].rearrange("(n p c f) -> n p c f", p=128, c=8, f=f)

    io_pool = ctx.enter_context(tc.tile_pool(name="io", bufs=2))
    pl_pool = ctx.enter_context(tc.tile_pool(name="planes", bufs=2))
    tmp_pool = ctx.enter_context(tc.tile_pool(name="tmp", bufs=2))

    for i in range(nt):
        At = io_pool.tile([128, f, 4], F32, tag="A")
        Bt = io_pool.tile([128, f, 4], F32, tag="B")
        nc.sync.dma_start(At[:], Av[i])
        nc.sync.dma_start(Bt[:], Bv[i])

        # Deinterleave input bit j into a bf16 plane pre-scaled by 2^j.
        # Slots 0..3 = A0..A3 (scaled 1,2,4,8); 4..7 = B0..B3.
        Dp = pl_pool.tile([128, 8, f], BF16, tag="D")
        for j in range(4):
            # one plane per input on DVE to offload ACT slightly
            eng = nc.vector if j == 0 else nc.scalar
            if j == 0:
                nc.vector.tensor_copy(Dp[:, 0, :], At[:, :, 0])
                nc.vector.tensor_copy(Dp[:, 4, :], Bt[:, :, 0])
            else:
                s = float(2 ** j)
                nc.scalar.activation(Dp[:, j, :], At[:, :, j], AF.Copy, bias=0.0, scale=s)
                nc.scalar.activation(Dp[:, 4 + j, :], Bt[:, :, j], AF.Copy, bias=0.0, scale=s)

        # a,b via tt-add tree on pre-scaled planes (all bf16, exact)
        u = tmp_pool.tile([128, f], BF16, tag="u")
        v = tmp_pool.tile([128, f], BF16, tag="v")
        a = tmp_pool.tile([128, f], BF16, tag="a")
        nc.vector.tensor_tensor(u[:], Dp[:, 0, :], Dp[:, 1, :], ALU.add)
        nc.vector.tensor_tensor(v[:], Dp[:, 2, :], Dp[:, 3, :], ALU.add)
        nc.vector.tensor_tensor(a[:], u[:], v[:], ALU.add)
        u2 = tmp_pool.tile([128, f], BF16, tag="u2")
        v2 = tmp_pool.tile([128, f], BF16, tag="v2")
        b = tmp_pool.tile([128, f], BF16, tag="b")
        nc.vector.tensor_tensor(u2[:], Dp[:, 4, :], Dp[:, 5, :], ALU.add)
        nc.vector.tensor_tensor(v2[:], Dp[:, 6, :], Dp[:, 7, :], ALU.add)
        nc.vector.tensor_tensor(b[:], u2[:], v2[:], ALU.add)

        p = tmp_pool.tile([128, f], BF16, tag="p")
        nc.vector.tensor_mul(p[:], a[:], b[:])

        # bits 7..1: compare-subtract chain, contiguous bf16 planes out
        Pt = io_pool.tile([128, 8, f], BF16, tag="O")
        r = p
        for k in range(7, 0, -1):
            nc.vector.tensor_scalar(Pt[:, k, :], r[:], float(2 ** k), None, ALU.is_ge)
            if k > 1:
                rn = tmp_pool.tile([128, f], BF16, tag=f"r{k % 2}")
                nc.vector.scalar_tensor_tensor(
                    rn[:], Pt[:, k, :], float(-(2 ** k)), r[:], ALU.mult, ALU.add
                )
                r = rn
        # bit 0 = A0 AND B0 = A0*B0 (planes 0 and 4 are unscaled)
        nc.vector.tensor_mul(Pt[:, 0, :], Dp[:, 0, :], Dp[:, 4, :])
        nc.sync.dma_start(Ov[i], Pt[:])


def build(rows: int = R, f: int = F) -> bass.Bass:
    nc = bacc.Bacc()
    Ah = nc.declare_dram_parameter("A", [rows, 4], F32, isOutput=False)
    Bh = nc.declare_dram_parameter("B", [rows, 4], F32, isOutput=False)
    Oh = nc.declare_dram_parameter("O", [rows * 8], BF16, isOutput=True)
    with tile.TileContext(nc) as tc:
        with ExitStack() as ctx:
            emit_multiplier(ctx, tc, Ah, Bh, Oh, rows, f)
    nc.finalize()
    return nc


def _run(A: np.ndarray, B: np.ndarray, trace: bool = False, tmpdir: str | None = None):
    A = np.ascontiguousarray(np.asarray(A), dtype=np.float32)
    B = np.ascontiguousarray(np.asarray(B), dtype=np.float32)
    assert A.shape == (N_FULL, 4) and B.shape == (N_FULL, 4), (A.shape, B.shape)

    nc = build(R, F)
    in_maps = [
        {"A": A[i * R:(i + 1) * R], "B": B[i * R:(i + 1) * R]}
        for i in range(N_CORES)
    ]
    kres = run_bass_kernel_spmd(
        nc, in_maps, list(range(N_CORES)), trace=trace, tmpdir=tmpdir
    )
    out = np.empty((N_FULL, 8), dtype=np.float32)
    nt = R // ROWS_PER_TILE
    for i in range(N_CORES):
        planes = kres.results[i]["O"].reshape(nt, 128, 8, F)
        # [nt, 128, 8, f] -> rows (nt, 128, f) x bits
        shard = np.transpose(planes, (0, 1, 3, 2)).reshape(R, 8)
        out[i * R:(i + 1) * R] = shard  # bf16 -> f32, exact for 0/1
    return out, kres


def kernel(A: np.ndarray, B: np.ndarray) -> np.ndarray:
    out, _ = _run(A, B, trace=False)
    return out
